# revision 23
# baseline (speedup 1.0000x reference)
"""Trainium2 (8-core) kernel for a GQA attention layer with rotary embeddings.

Reference computation (N=2048 tokens, D=1024, H=16 q-heads, KV=4 kv-heads, HD=64):
    xq = rope(x @ Wq), xk = rope(x @ Wk), xv = x @ Wv
    out = softmax(xq xk^T / sqrt(HD)) @ xv   (full attention, GQA)
    return out @ Wo

Sharding: 2 q-heads + their shared kv-head per core. Attention runs in a
transposed (S^T) layout, token-block-major, with the two q-heads row-packed
into one concurrent PE pair per k-block (q left in its natural [h0; h1] row
layout, so only k needs row duplication — one DMA). Softmax exp alternates
between the Scalar engine (table exp, 10/16 blocks) and a custom DVE op
(cubic^4 approximation, 6/16); the PV stationary carries V plus a single
ones column so the softmax denominator falls out of the same matmul. The
rope half-swap uses DMA for q and DVE stream_shuffle for k. Token blocks
are staged into interleaved-destination AllToAlls (one per 1024 tokens,
fired mid-attention behind an early CC-warmup barrier collective), so the
output projection overlaps the attention tail; each core finishes tokens
{512b + 64r .. +64} re-interleaved by the host wrapper.
"""

import numpy as np
import ml_dtypes

import concourse.bass as bass
import concourse.tile as tile
from concourse.masks import make_identity
from concourse import bacc, mybir
from concourse.bass_utils import run_bass_kernel_spmd

bf16 = ml_dtypes.bfloat16
BF16 = mybir.dt.bfloat16
F32 = mybir.dt.float32

N, D, H, KV, HD = 2048, 1024, 16, 4, 64
NC = 8
HPC = H // NC            # q-heads per core = 2
NTB = 4                  # token blocks of 512
TBW = N // NTB           # 512
NKB = N // 128           # 16 k-blocks of 128 tokens
NCHUNK = D // 128        # 8 contraction chunks

# exp(4y) ~= (1 + c1 y + c2 y^2 + c3 y^3)^4 minimax-fit on y in [-0.7, 0.7];
# scores/32 land in [-0.65, 0.65]. Max relative error ~0.65%.
EXP_C1, EXP_C2, EXP_C3 = 1.00305985, 0.51686418, 0.16136205

# k-blocks whose exp runs on the DVE (custom op); the rest go to ACT.
DVE_KB = frozenset({2, 4, 6, 9, 11, 14})

IDENT32 = list(range(32))


# ---------------------------------------------------------------- custom DVE op
def _register_exp_op():
    import concourse.dve_ops as dve_ops_mod
    from concourse.dve_spec import Spec, Src0, C0, C1, C2, One, sq, lower
    from concourse.dve_uop import DveOpSpec

    name = "EXP_CUBIC_POW4_ANT"
    for op in dve_ops_mod.OPS:
        if op.name == name:
            return op

    y = Src0
    p = ((C2 * y + C1) * y + C0) * y + One
    body = sq(sq(p))

    def ref(in0, in1, s0, s1, imm2):
        pp = ((imm2 * in0 + s1) * in0 + s0) * in0 + 1.0
        return (pp * pp) * (pp * pp)

    spec = Spec(body=body, reference=ref)
    row = dve_ops_mod._CUSTOM_DVE_ROW_BASE + len(dve_ops_mod.OPS)
    shas = {}
    for ver in ("v3", "v4"):
        try:
            uops = lower(spec, ver=ver)
            shas[ver] = DveOpSpec(name=name, opcode=row, uops=uops, rd1_en=False).sha(
                ver
            )
        except Exception:
            pass
    op = dve_ops_mod.DveOp(name, spec, subdim=False, uops_sha=shas)
    dve_ops_mod.OPS.append(op)
    dve_ops_mod.CUSTOM_DVE_SPECS[name] = spec
    dve_ops_mod._SUB_OPCODE_FOR_NAME[name] = row
    return op


EXP_OP = _register_exp_op()


# ---------------------------------------------------------------- device kernel
def _build_nc(dbg=False):
    nc = bacc.Bacc(
        "TRN2", target_bir_lowering=False, debug=False, num_devices=NC
    )
    xt = nc.dram_tensor("xt", [128, NTB, NCHUNK, TBW], BF16, kind="ExternalInput").ap()
    wq = nc.dram_tensor("wq", [128, NCHUNK, 128], BF16, kind="ExternalInput").ap()
    wkv = nc.dram_tensor("wkv", [128, NCHUNK, 128], BF16, kind="ExternalInput").ap()
    wo = nc.dram_tensor("wo", [128, NCHUNK, D], BF16, kind="ExternalInput").ap()
    cosf = nc.dram_tensor("cosf", [128, N], BF16, kind="ExternalInput").ap()
    sinf = nc.dram_tensor("sinf", [128, N], BF16, kind="ExternalInput").ap()
    out = nc.dram_tensor("out", [NTB, 64, D], F32, kind="ExternalOutput").ap()
    dbg_aps = None
    if dbg:
        dbg_aps = {
            name: nc.dram_tensor(f"dbg_{name}", shape, BF16, kind="ExternalOutput").ap()
            for name, shape in [
                ("qtd", [128, N]), ("ktd", [128, N]), ("vp", [128, NKB * 128]),
                ("ofin", [128, N]), ("og", [128, NCHUNK, NTB, 64]),
            ]
        }

    with tile.TileContext(nc) as tc:
        _emit(nc, tc, xt, wq, wkv, wo, cosf, sinf, out, dbg_aps)
    nc.compile()
    return nc


def _emit(nc, tc, xt, wq, wkv, wo, cosf, sinf, out, dbg_aps=None):
    fexp = mybir.ActivationFunctionType.Exp

    with (
        tc.tile_pool(name="persist", bufs=1) as pp,
        tc.tile_pool(name="work", bufs=3) as wp,
        tc.tile_pool(name="big_ps", bufs=2, space="PSUM") as big_ps,
        tc.tile_pool(name="u_ps", bufs=4, space="PSUM") as u_ps,
        tc.tile_pool(name="ptp", bufs=6) as ptp,
        tc.tile_pool(name="dram", bufs=1, space="DRAM") as dram,
    ):
        # ---- persistent SBUF tensors
        xall = pp.tile([128, NTB, NCHUNK, TBW], BF16, tag="xall")
        wq_sb = pp.tile([128, NCHUNK, 128], BF16, tag="wq")
        wkv_sb = pp.tile([128, NCHUNK, 128], BF16, tag="wkv")
        wo_sb = pp.tile([128, NCHUNK, D], BF16, tag="wo")
        cos_sb = pp.tile([128, N], BF16, tag="cos")
        sin_sb = pp.tile([128, N], BF16, tag="sin")
        qtd = pp.tile([128, N], BF16, tag="qtd")      # q^T: rows = [h0(r,i) | h1(r,i)]
        ktd = pp.tile([128, N], BF16, tag="ktd")      # k^T: rows 0:63 = k, 64:127 dup
        vp = pp.tile([128, NKB * 128], BF16, tag="vp")  # [V(64) | ones] per k-block
        ofin = pp.tile([128, N], BF16, tag="ofin")    # normalized attention out^T
        og = pp.tile([128, NCHUNK, NTB, 64], BF16, tag="og")  # gathered o for oproj
        ident = pp.tile([128, 128], BF16, tag="ident")

        # Sync HWDGE ring drains FIFO: x blocks arrive in projection order.
        # cos/sin/wo ride the ACT ring in parallel (needed later than xt0).
        nc.sync.dma_start(wq_sb[:], wq)
        nc.sync.dma_start(wkv_sb[:], wkv)
        nc.sync.dma_start(xall[:, 0, 0:4], xt[:, 0, 0:4])
        nc.sync.dma_start(xall[:, 0, 4:8], xt[:, 0, 4:8])
        for tb in range(1, NTB):
            nc.sync.dma_start(xall[:, tb], xt[:, tb])
        nc.scalar.dma_start(cos_sb[:], cosf)
        nc.scalar.dma_start(sin_sb[:], sinf)
        nc.scalar.dma_start(wo_sb[:], wo)
        make_identity(nc, ident[:])
        nc.gpsimd.memset(vp[:], 1.0)

        # early dummy barrier collective: absorbs cross-core start skew and
        # the CC-pipeline warmup while the input DMAs stream in
        barrier_in = dram.tile([NC, 16, 1], BF16, tag="barrier_in")
        barrier_out = dram.tile([NC, 16, 1], BF16, tag="barrier_out")
        nc.gpsimd.collective_compute(
            "AllToAll",
            mybir.AluOpType.bypass,
            replica_groups=[list(range(NC))],
            ins=[barrier_in.opt()],
            outs=[barrier_out.opt()],
        )

        # ---- projections + rope (stream_shuffle half-swap) + V transpose
        for tb in range(NTB):
            ts_ = slice(tb * TBW, (tb + 1) * TBW)
            pqkv = big_ps.tile([128, 1024], F32, tag="big")
            for c in range(NCHUNK):
                nc.tensor.matmul(
                    pqkv[:, 0:512], wq_sb[:, c, :], xall[:, tb, c, :],
                    start=(c == 0), stop=(c == NCHUNK - 1),
                )
            for c in range(NCHUNK):
                nc.tensor.matmul(
                    pqkv[:, 512:1024], wkv_sb[:, c, :], xall[:, tb, c, :],
                    start=(c == 0), stop=(c == NCHUNK - 1),
                )
            qk = wp.tile([128, 1024], BF16, tag="qksb")  # [q | kv]
            nc.scalar.copy(qk[:], pqkv[:])

            # rope. k first: the attention gate is ktd across all blocks, so
            # its chain leads; q' = q*cos + swap(q)*[-s;+s], swap via DMA for
            # q (Sync is idle) and stream_shuffle for k.
            ksw = wp.tile([64, 512], BF16, tag="ksw")
            nc.vector.stream_shuffle(ksw[0:32, :], qk[32:64, 512:1024], IDENT32)
            nc.vector.stream_shuffle(ksw[32:64, :], qk[0:32, 512:1024], IDENT32)
            t1k = wp.tile([64, 512], BF16, tag="ropet1k")
            nc.vector.tensor_mul(t1k[:], qk[0:64, 512:1024], cos_sb[0:64, ts_])
            t2k = wp.tile([64, 512], BF16, tag="ropet2k")
            nc.vector.tensor_mul(t2k[:], ksw[:], sin_sb[0:64, ts_])
            nc.vector.tensor_add(ktd[0:64, ts_], t1k[:], t2k[:])

            qsw = wp.tile([128, 512], BF16, tag="qsw")
            for b in (0, 64):
                nc.sync.dma_start(qsw[b: b + 32, :], qk[b + 32: b + 64, 0:512])
                nc.sync.dma_start(qsw[b + 32: b + 64, :], qk[b: b + 32, 0:512])
            t1 = wp.tile([128, 512], BF16, tag="ropet1")
            nc.vector.tensor_mul(t1[:], qk[:, 0:512], cos_sb[:, ts_])
            t2 = wp.tile([128, 512], BF16, tag="ropet2")
            nc.vector.tensor_mul(t2[:], qsw[:], sin_sb[:, ts_])
            nc.vector.tensor_add(qtd[:, ts_], t1[:], t2[:])

            # V natural: transpose v^T (rows 64:127 of kv half) per 128-col block
            for j in range(4):
                kb = tb * 4 + j
                vt = u_ps.tile([128, 64], BF16, tag="u")
                nc.tensor.transpose(
                    vt[0:128, 0:64],
                    qk[64:128, 512 + j * 128: 512 + (j + 1) * 128],
                    ident[64:128, 64:128],
                )
                nc.vector.tensor_copy(vp[:, kb * 128: kb * 128 + 64], vt[:, 0:64])

        # duplicate k rows for the row-packed S pairs
        nc.sync.dma_start(ktd[64:128, :], ktd[0:64, :])

        # ---- attention + interleaved AllToAll + output projection
        def _oproj_pass(G):
            osb = wp.tile([128, 1024], F32, tag="osb")
            for n_ in range(2):
                po = u_ps.tile([128, 512], F32, tag="u")
                for c in range(NCHUNK):
                    nc.tensor.matmul(
                        po[:],
                        og[:, c, 2 * G: 2 * G + 2, :],
                        wo_sb[:, c, n_ * 512:(n_ + 1) * 512],
                        start=(c == 0), stop=(c == NCHUNK - 1),
                    )
                nc.scalar.copy(osb[:, n_ * 512:(n_ + 1) * 512], po[:])
                nc.sync.dma_start(
                    out[2 * G: 2 * G + 2, :, n_ * 512:(n_ + 1) * 512].rearrange(
                        "b t d -> (b t) d"
                    ),
                    osb[:, n_ * 512:(n_ + 1) * 512],
                )

        a2a_ins = []
        for tb in range(NTB):
            ts_ = slice(tb * TBW, (tb + 1) * TBW)
            acc0 = u_ps.tile([128, 512], F32, tag="u")
            acc1 = u_ps.tile([128, 512], F32, tag="u")
            for kb in range(NKB):
                ks_ = slice(kb * 128, (kb + 1) * 128)
                st = big_ps.tile([128, 1024], F32, tag="big")
                nc.tensor.matmul(
                    st[:, 0:512], ktd[0:64, ks_], qtd[0:64, ts_],
                    start=True, stop=True,
                )
                nc.tensor.matmul(
                    st[:, 512:1024], ktd[64:128, ks_], qtd[64:128, ts_],
                    start=True, stop=True,
                )
                pt = ptp.tile([128, 1024], BF16, tag="pt")
                if kb in DVE_KB:
                    nc.vector._custom_dve(
                        EXP_OP, out=pt[:], in0=st[:],
                        s0=EXP_C1, s1=EXP_C2, imm2=EXP_C3,
                    )
                else:
                    nc.scalar.activation(pt[:], st[:], fexp, scale=4.0)
                # 65 weight columns: V(64) + one ones column for the sums row;
                # keeps the LDWEIGHTS at half cost vs a 128-col stationary
                nc.tensor.matmul(
                    acc0[0:65, :], vp[:, kb * 128: kb * 128 + 65], pt[:, 0:512],
                    start=(kb == 0), stop=(kb == NKB - 1),
                )
                nc.tensor.matmul(
                    acc1[0:65, :], vp[:, kb * 128: kb * 128 + 65], pt[:, 512:1024],
                    start=(kb == 0), stop=(kb == NKB - 1),
                )

            if tb == 3:
                # pass 0 sits between tb3's attention matmuls and its norm:
                # collective 0 has had two token blocks of slack to complete
                _oproj_pass(0)

            # normalize: V-out on rows 0:63, sums on row 64. The sums row goes
            # PSUM -> SBUF (ACT) -> partition 0 (DMA), then recip + GpSimd
            # partition-broadcast feed a PSUM-direct multiply.
            for h, acc in ((0, acc0), (1, acc1)):
                asb = wp.tile([65, 512], F32, tag="asb")
                nc.scalar.copy(asb[64:65, :], acc[64:65, :])
                sdown = wp.tile([1, 512], F32, tag="sdown")
                nc.sync.dma_start(sdown[:], asb[64:65, :])
                rs = wp.tile([1, 512], F32, tag="rsum")
                nc.vector.reciprocal_approx_fast(rs[:], sdown[:])
                rsb = wp.tile([64, 512], F32, tag="rsb")
                nc.gpsimd.partition_broadcast(rsb[:], rs[:])
                if h == 0:
                    nc.vector.tensor_mul(ofin[0:64, ts_], acc[0:64, :], rsb[:])
                else:
                    ot = wp.tile([64, 512], BF16, tag="onorm")
                    nc.vector.tensor_mul(ot[:], acc[0:64, :], rsb[:])

            # stage this token block: 64-token strip j -> slot j, half tb%2;
            # one AllToAll per block pair
            P = tb // 2
            if tb % 2 == 0:
                a2a_in = dram.tile([NC, 128, 2, 64], BF16, tag=f"a2ain{P}")
                a2a_ins.append(a2a_in)
            else:
                a2a_in = a2a_ins[P]
            nc.sync.dma_start(
                a2a_in[:, 0:64, tb % 2, :].rearrange("j p t -> p j t"),
                ofin[0:64, ts_],
            )
            nc.sync.dma_start(
                a2a_in[:, 64:128, tb % 2, :].rearrange("j p t -> p j t"), ot[:]
            )
            if tb % 2 == 1:
                a2a_out = dram.tile([NC, 128, 2, 64], BF16, tag=f"a2aout{P}")
                nc.gpsimd.collective_compute(
                    "AllToAll",
                    mybir.AluOpType.bypass,
                    replica_groups=[list(range(NC))],
                    ins=[a2a_in.opt()],
                    outs=[a2a_out.opt()],
                )
                nc.sync.dma_start(
                    og[:, :, 2 * P: 2 * P + 2, :],
                    a2a_out.rearrange("c p i t -> p c i t"),
                )

        _oproj_pass(1)

        if dbg_aps is not None:
            nc.sync.dma_start(dbg_aps["qtd"], qtd[:])
            nc.sync.dma_start(dbg_aps["ktd"], ktd[:])
            nc.sync.dma_start(dbg_aps["vp"], vp[:])
            nc.sync.dma_start(dbg_aps["ofin"], ofin[:])
            nc.sync.dma_start(dbg_aps["og"], og[:])


_NC_CACHE = None


def _get_nc():
    global _NC_CACHE
    if _NC_CACHE is None:
        _NC_CACHE = _build_nc()
    return _NC_CACHE


# ---------------------------------------------------------------- host wrapper
_ROPE_PERM = np.concatenate([np.arange(0, HD, 2), np.arange(1, HD, 2)])


def _chunked(w):
    """(D, F) -> (128, D//128, F) so [p, c, f] = w[128c+p, f]."""
    return np.ascontiguousarray(
        w.reshape(D // 128, 128, -1).transpose(1, 0, 2)
    )


def _prep_inputs(x, freqs_cos, freqs_sin, Wq, Wk, Wv, Wo):
    x = np.asarray(x, np.float32)
    Wq = np.asarray(Wq, np.float32)
    Wk = np.asarray(Wk, np.float32)
    Wv = np.asarray(Wv, np.float32)
    Wo = np.asarray(Wo, np.float32)
    cos = np.asarray(freqs_cos, np.float32)
    sin = np.asarray(freqs_sin, np.float32)

    # xt[p, tb, c, n] = x[512*tb + n, 128*c + p]
    xtv = np.ascontiguousarray(
        x.T.reshape(NCHUNK, 128, NTB, TBW).transpose(1, 2, 0, 3)
    ).astype(bf16)
    cosf = np.tile(cos.T, (4, 1)).astype(bf16)
    sinf = np.tile(np.concatenate([-sin.T, sin.T], axis=0), (2, 1)).astype(bf16)
    wo_dev = _chunked(Wo).astype(bf16)

    in_maps = []
    for r in range(NC):
        h0, h1 = 2 * r, 2 * r + 1
        g = r // 2
        # q pre-scaled by 1/32: folds the 1/sqrt(HD)=1/8 softmax scale and the
        # /4 for the (cubic)^4 exp decomposition into the weights.
        wq_core = np.concatenate(
            [
                Wq[:, 64 * h0 + _ROPE_PERM],
                Wq[:, 64 * h1 + _ROPE_PERM],
            ],
            axis=1,
        ) * (1.0 / 32.0)
        wkv_core = np.concatenate(
            [Wk[:, 64 * g + _ROPE_PERM], Wv[:, 64 * g: 64 * g + HD]], axis=1
        )
        in_maps.append(
            {
                "xt": xtv,
                "wq": _chunked(wq_core).astype(bf16),
                "wkv": _chunked(wkv_core).astype(bf16),
                "wo": wo_dev,
                "cosf": cosf,
                "sinf": sinf,
            }
        )
    return in_maps


def _run(inputs, trace=False, dbg=False, **spmd_kwargs):
    in_maps = _prep_inputs(**inputs)
    nc = _build_nc(dbg=True) if dbg else _get_nc()
    res = run_bass_kernel_spmd(
        nc, in_maps, core_ids=list(range(NC)), trace=trace, **spmd_kwargs
    )
    # core r computed tokens {512*b + 64*r + t} for b in 0..3, t in 0..63
    full = np.empty((N, D), np.float32)
    for r in range(NC):
        o = res.results[r]["out"].reshape(NTB, 64, D)
        for b in range(NTB):
            full[512 * b + 64 * r: 512 * b + 64 * r + 64] = o[b]
    return full, res


def kernel(**inputs):
    out, _ = _run(inputs, trace=False)
    return out
